# revision 1
# baseline (speedup 1.0000x reference)
"""DCGRU cell on 8 Trainium2 NeuronCores.

Sharding: data-parallel over batch B=64 -> 8 batches per core. Supports and
weights replicated; everything per-core is local (no collectives).

Per-core computation (Bl=8 local batches, N=4096 nodes, F=66 feats):
  Graph diffusion x_m = T_m(s) x0 runs as dense matmuls on the PE with the
  per-core x tile [128, nt, 528] as the *stationary* operand and transposed
  support tiles streamed as the *moving* operand, producing the diffused
  mats directly in feature-major layout [c=(b,f), n] (what the output
  projection wants). Chain mats (x1, x3) are PE-transposed back to
  graph-major for the next diffusion step; terminal mats (x2, x4) fuse the
  Chebyshev combine 2*s@x1 - x0 into the PSUM evacuation. Feature-major
  mats are staged through per-n-chunk DRAM stash tensors (SBUF cannot hold
  all five), and the output projection for chunk k is emitted right after
  the final diffusion step's chunk k so it overlaps on the PE. float32r
  matmuls give ~1e-4 relative error at full (1 cycle/row) PE rate.

Feature order inside a batch block is [hx(64), inputs(2)] (a host-side
permutation of x columns and weight rows) so every partition slice used on
the device is 32-aligned.
"""
import sys

import ml_dtypes
import numpy as np

sys.path.insert(0, "/opt/trn_rl_repo")

from concourse import bacc, mybir, tile  # noqa: E402
from concourse.bass_utils import run_bass_kernel_spmd  # noqa: E402

B = 64
N = 4096
U = 64
IN_DIM = 2
F = U + IN_DIM          # 66, feature order [hx, inputs]
NCORES = 8
BL = B // NCORES        # 8
FB = F * BL             # 528
NT = N // 128           # 32 n-tiles
JT = N // 128           # 32 j-tiles
NCH = 8                 # n-chunks of 512
CHW = N // NCH          # 512
CTS = [(0, 128), (128, 128), (256, 128), (384, 128), (512, 16)]  # c-tiles of FB

F32 = mybir.dt.float32
F32R = mybir.dt.float32r
BF16 = mybir.dt.bfloat16
SPMM_BF16 = True            # bf16 spmm operands (x graph-major + supports)
DT_S = BF16 if SPMM_BF16 else F32R
DT_X = BF16 if SPMM_BF16 else F32R
MULT = mybir.AluOpType.mult
SUBTRACT = mybir.AluOpType.subtract
SIGMOID = mybir.ActivationFunctionType.Sigmoid
TANH = mybir.ActivationFunctionType.Tanh

_BUILD_CACHE = {}
_NAME_N = [0]


def _nm(base):
    _NAME_N[0] += 1
    return f"{base}_{_NAME_N[0]}"


def _emit_spmm(nc, pools, sT_d, sup, src, terminal, stash_l, m_idx,
               xg_dst, ident, post_nchunk=None):
    """One diffusion step: out = (supports[sup] @ src).T, feature-major.

    src: graph-major SBUF tile [128, NT, FB] (stationary operand).
    stash_l: per-nchunk DRAM tiles [4, FB, CHW]; this mat writes index m_idx.
    terminal=False: also PE-transpose-back to xg_dst for the next chain step.
    The Chebyshev combine (2*y - x0) is folded into the projection weights
    host-side, so terminal mats stash the raw spmm output.
    post_nchunk: optional callback emitted after each chunk's evacuations.
    """
    pps, pst, pstage = pools["pps"], pools["pst"], pools["pstage"]
    for nch in range(NCH):
        n0 = nch * CHW
        psums = [pps.tile([128, CHW], F32, tag="ps", name=_nm("sp")) for _ in range(5)]
        for jtg in range(16):
            st = pst.tile([128, 2, CHW], DT_S, tag="st", name=_nm("st"))
            nc.sync.dma_start(
                st[:],
                sT_d[sup, jtg * 256:(jtg + 1) * 256, n0:n0 + CHW].rearrange(
                    "(t p) n -> p t n", p=128
                ),
            )
            for j2 in range(2):
                jt = jtg * 2 + j2
                for ci, (c0, cw) in enumerate(CTS):
                    nc.tensor.matmul(
                        psums[ci][0:cw, :],
                        src[:, jt, c0:c0 + cw],
                        st[:, j2, :],
                        start=(jt == 0),
                        stop=(jt == JT - 1),
                    )
        for ci, (c0, cw) in enumerate(CTS):
            stg = pstage.tile([128, CHW], F32R, tag="stage", name=_nm("stg"))
            if ci % 2 == 0:
                nc.vector.tensor_copy(stg[0:cw, :], psums[ci][0:cw, :])
            else:
                nc.scalar.copy(stg[0:cw, :], psums[ci][0:cw, :])
            nc.sync.dma_start(stash_l[nch][m_idx, c0:c0 + cw, :], stg[0:cw, :])
            if not terminal:
                for blk in range(4):
                    tp = pps.tile([128, 128], F32R, tag="tp", bufs=1, name=_nm("tp"))
                    nc.tensor.transpose(
                        tp[0:128, 0:cw],
                        stg[0:cw, blk * 128:(blk + 1) * 128],
                        ident[0:cw, 0:cw],
                    )
                    nc.scalar.copy(xg_dst[:, nch * 4 + blk, c0:c0 + cw], tp[0:128, 0:cw])
        if post_nchunk is not None:
            post_nchunk(nch)


def _emit_proj_nchunk(nc, pools, g, nch, w, bias, m0_at, hx_d, stash_l,
                      xt02_l, u_l, out_d, xg2, ident):
    """Projection + gating for gconv g at one n-chunk (all 8 local batches)."""
    pmov, pps, psig, ps2, pgate = (
        pools["pmov"], pools["pps"], pools["psig"], pools["ps2"], pools["pgate"],
    )
    O = 128 if g == 0 else 64
    n0 = nch * CHW
    for b in range(BL):
        mv0 = pmov.tile([F, CHW], F32R, tag="mov0", bufs=4, name=_nm("mv0"))
        nc.sync.dma_start(mv0[:], m0_at(nch)[b * F:(b + 1) * F, :])
        mv4 = pmov.tile([F, 4, CHW], F32R, tag="mov4", bufs=4, name=_nm("mv4"))
        nc.sync.dma_start(
            mv4[:],
            stash_l[nch][:, b * F:(b + 1) * F, :].rearrange("m f n -> f m n"),
        )
        pp = pps.tile([128, CHW], F32, tag="ps", name=_nm("pp"))
        nc.tensor.matmul(pp[0:O, :], w[:, 0, :], mv0[:], start=True, stop=False)
        for m in range(1, 5):
            nc.tensor.matmul(
                pp[0:O, :], w[:, m, :], mv4[:, m - 1, :],
                start=False, stop=(m == 4),
            )
        if g == 0:
            sig = psig.tile([128, CHW], F32, tag="sig", name=_nm("sig"))
            nc.scalar.activation(sig[:], pp[:], SIGMOID, bias=bias[:])
            s2 = ps2.tile([F, CHW], F32R, tag="s2", name=_nm("s2"))
            # rows 0:64 = r * hx (feature-major), rows 64:66 = inputs
            nc.vector.tensor_mul(s2[0:64, :], sig[0:64, :], mv0[0:64, :])
            nc.vector.tensor_copy(s2[64:66, :], mv0[64:66, :])
            nc.sync.dma_start(xt02_l[nch][b * F:(b + 1) * F, :], s2[:])
            nc.sync.dma_start(u_l[nch][b, :, :], sig[64:128, :])
            for blk in range(4):
                tp = pps.tile([128, 128], F32R, tag="tp", bufs=1, name=_nm("tp"))
                nc.tensor.transpose(
                    tp[0:128, 0:F], s2[:, blk * 128:(blk + 1) * 128],
                    ident[0:F, 0:F],
                )
                nc.scalar.copy(xg2[:, nch * 4 + blk, b * F:(b + 1) * F], tp[0:128, 0:F])
        else:
            ct = pgate.tile([64, CHW], F32, tag="ct", name=_nm("ct"))
            nc.scalar.activation(ct[:], pp[0:64, :], TANH, bias=bias[:])
            ut = pgate.tile([64, CHW], F32, tag="ut", name=_nm("ut"))
            nc.sync.dma_start(ut[:], u_l[nch][b, :, :])
            hxt = pgate.tile([64, CHW], F32R, tag="hxt", name=_nm("hxt"))
            nc.sync.dma_start(hxt[:], hx_d[b * F:b * F + 64, n0:n0 + CHW])
            t1 = pgate.tile([64, CHW], F32, tag="t1", name=_nm("t1"))
            nc.vector.tensor_sub(t1[:], hxt[:], ct[:])
            nc.vector.tensor_mul(t1[:], ut[:], t1[:])
            nc.vector.tensor_add(t1[:], t1[:], ct[:])
            nc.sync.dma_start(out_d[b, :, n0:n0 + CHW], t1[:])


def _build(reps=1):
    if reps in _BUILD_CACHE:
        return _BUILD_CACHE[reps]
    nc = bacc.Bacc("TRN2", target_bir_lowering=False, debug=False)

    sT_d = nc.dram_tensor("sT", [2, N, N], DT_S, kind="ExternalInput").ap()
    xg0_d = nc.dram_tensor("xg0", [N, FB], DT_X, kind="ExternalInput").ap()
    xt0c_d = nc.dram_tensor("xt0c", [FB, N], F32R, kind="ExternalInput").ap()
    w1_d = nc.dram_tensor("w1", [F, 5, 2 * U], F32R, kind="ExternalInput").ap()
    b1_d = nc.dram_tensor("b1", [2 * U, 1], F32, kind="ExternalInput").ap()
    w2_d = nc.dram_tensor("w2", [F, 5, U], F32R, kind="ExternalInput").ap()
    b2_d = nc.dram_tensor("b2", [U, 1], F32, kind="ExternalInput").ap()
    id_d = nc.dram_tensor("ident", [128, 128], F32R, kind="ExternalInput").ap()
    out_d = nc.dram_tensor("outT", [BL, U, N], F32, kind="ExternalOutput").ap()

    with tile.TileContext(nc) as tc:
        with (
            tc.tile_pool(name="dram", bufs=1, space="DRAM") as dram,
            tc.tile_pool(name="pw", bufs=1) as pw,
            tc.tile_pool(name="pxg", bufs=1) as pxg,
            tc.tile_pool(name="pst", bufs=6) as pst,
            tc.tile_pool(name="pstage", bufs=6) as pstage,
            tc.tile_pool(name="pmov", bufs=3) as pmov,
            tc.tile_pool(name="ps2", bufs=3) as ps2,
            tc.tile_pool(name="psig", bufs=3) as psig,
            tc.tile_pool(name="pgate", bufs=3) as pgate,
            tc.tile_pool(name="pps", bufs=7, space="PSUM") as pps,
        ):
            pools = dict(
                pst=pst, pstage=pstage, pmov=pmov, ps2=ps2,
                psig=psig, pgate=pgate, pps=pps,
            )
            # per-n-chunk DRAM scratch (fine dep granularity -> phase overlap)
            stash1 = [dram.tile([4, FB, CHW], F32R, name=_nm("stA")) for _ in range(NCH)]
            stash2 = [dram.tile([4, FB, CHW], F32R, name=_nm("stB")) for _ in range(NCH)]
            xt02_l = [dram.tile([FB, CHW], F32R, name=_nm("xt02")) for _ in range(NCH)]
            u_l = [dram.tile([BL, U, CHW], F32, name=_nm("ud")) for _ in range(NCH)]

            w1 = pw.tile([F, 5, 2 * U], F32R, tag="w1")
            nc.sync.dma_start(w1[:], w1_d)
            w2 = pw.tile([F, 5, U], F32R, tag="w2")
            nc.sync.dma_start(w2[:], w2_d)
            b1 = pw.tile([2 * U, 1], F32, tag="b1")
            nc.sync.dma_start(b1[:], b1_d)
            b2 = pw.tile([U, 1], F32, tag="b2")
            nc.sync.dma_start(b2[:], b2_d)
            ident = pw.tile([128, 128], F32R, tag="ident")
            nc.sync.dma_start(ident[:], id_d)

            xt0c_at = lambda nch: xt0c_d[:, nch * CHW:(nch + 1) * CHW]  # noqa: E731
            xt02_at = lambda nch: xt02_l[nch][:, :]  # noqa: E731

            for _rep in range(reps):
                xg0 = pxg.tile([128, NT, FB], DT_X, tag="xg", name=_nm("xg0"))
                nc.sync.dma_start(xg0[:], xg0_d.rearrange("(t p) c -> p t c", p=128))

                # ---- gconv 1 diffusion ----
                xc = pxg.tile([128, NT, FB], DT_X, tag="xc", name=_nm("xc"))
                _emit_spmm(nc, pools, sT_d, 0, xg0, False, stash1, 0, xc, ident)
                _emit_spmm(nc, pools, sT_d, 0, xc, True, stash1, 1, None, ident)
                xc2 = pxg.tile([128, NT, FB], DT_X, tag="xc", name=_nm("xc2"))
                _emit_spmm(nc, pools, sT_d, 1, xg0, False, stash1, 2, xc2, ident)

                # ---- x4 + gconv1 projection interleaved per n-chunk ----
                xg2 = pxg.tile([128, NT, FB], DT_X, tag="xg", name=_nm("xg2"))

                def proj1(nch, _xg2=xg2):
                    _emit_proj_nchunk(
                        nc, pools, 0, nch, w1, b1, xt0c_at, xt0c_d, stash1,
                        xt02_l, u_l, out_d, _xg2, ident,
                    )

                _emit_spmm(nc, pools, sT_d, 1, xc2, True, stash1, 3,
                           None, ident, post_nchunk=proj1)

                # ---- gconv 2 diffusion ----
                xc3 = pxg.tile([128, NT, FB], DT_X, tag="xc", name=_nm("xc3"))
                _emit_spmm(nc, pools, sT_d, 0, xg2, False, stash2, 0, xc3, ident)
                _emit_spmm(nc, pools, sT_d, 0, xc3, True, stash2, 1, None, ident)
                xc4 = pxg.tile([128, NT, FB], DT_X, tag="xc", name=_nm("xc4"))
                _emit_spmm(nc, pools, sT_d, 1, xg2, False, stash2, 2, xc4, ident)

                def proj2(nch):
                    _emit_proj_nchunk(
                        nc, pools, 1, nch, w2, b2, xt02_at, xt0c_d, stash2,
                        xt02_l, u_l, out_d, None, ident,
                    )

                _emit_spmm(nc, pools, sT_d, 1, xc4, True, stash2, 3,
                           None, ident, post_nchunk=proj2)

    nc.compile()
    _BUILD_CACHE[reps] = nc
    return nc


def _host_prep(inputs, hx, supports, ru_weights, ru_biases, gconv_weights, gconv_biases):
    """Build per-core input maps. Feature order inside a batch block: [hx, inputs]."""
    sT = np.ascontiguousarray(supports.transpose(0, 2, 1)).astype(np.float32)
    if SPMM_BF16:
        sT = sT.astype(ml_dtypes.bfloat16)
    x = np.concatenate(
        [hx.reshape(B, N, U), inputs.reshape(B, N, IN_DIM)], axis=2
    ).astype(np.float32)  # [B, N, F], feature order [hx, in]

    # weight rows are (f_orig, m) with f_orig order [in, hx]; permute to [hx, in]
    def prep_w(w, o):
        wr = w.reshape(F, 5, o).astype(np.float32)
        wr = np.concatenate([wr[IN_DIM:], wr[:IN_DIM]], axis=0).copy()
        # Chebyshev fold: x2 = 2*y2 - x0, x4 = 2*y4 - x0 with y = raw s@x1
        wr[:, 0] = wr[:, 0] - wr[:, 2] - wr[:, 4]
        wr[:, 2] = 2.0 * wr[:, 2]
        wr[:, 4] = 2.0 * wr[:, 4]
        return np.ascontiguousarray(wr)

    w1 = prep_w(ru_weights, 2 * U)
    w2 = prep_w(gconv_weights, U)
    b1 = np.ascontiguousarray(ru_biases.reshape(2 * U, 1)).astype(np.float32)
    b2 = np.ascontiguousarray(gconv_biases.reshape(U, 1)).astype(np.float32)
    ident = np.eye(128, dtype=np.float32)

    in_maps = []
    for c in range(NCORES):
        xb = x[c * BL:(c + 1) * BL]  # [BL, N, F]
        xg0 = np.ascontiguousarray(xb.transpose(1, 0, 2).reshape(N, FB))
        if SPMM_BF16:
            xg0 = xg0.astype(ml_dtypes.bfloat16)
        xt0c = np.ascontiguousarray(xb.transpose(0, 2, 1).reshape(FB, N))
        in_maps.append({
            "sT": sT, "xg0": xg0, "xt0c": xt0c,
            "w1": w1, "b1": b1, "w2": w2, "b2": b2, "ident": ident,
        })
    return in_maps


def kernel(inputs, hx, supports, ru_weights, ru_biases, gconv_weights, gconv_biases):
    nc = _build()
    in_maps = _host_prep(
        inputs, hx, supports, ru_weights, ru_biases, gconv_weights, gconv_biases
    )
    res = run_bass_kernel_spmd(nc, in_maps, list(range(NCORES))).results
    outs = []
    for c in range(NCORES):
        outT = res[c]["outT"]  # [BL, U, N]
        outs.append(outT.transpose(0, 2, 1).reshape(BL, N * U))
    return np.concatenate(outs, axis=0).astype(np.float32)



# revision 2
# speedup vs baseline: 1.1900x; 1.1900x over previous
"""DCGRU cell on 8 Trainium2 NeuronCores — fp8 DoubleRow spmm version.

Sharding: data-parallel over batch B=64 -> 8 batches per core. Supports and
weights replicated; everything per-core is local (no collectives).

Per-core computation (Bl=8 local batches, N=4096 nodes, F=66 feats):
  The 8 graph-diffusion spmms run as fp8-e4m3 DoubleRow matmuls (two j-tiles
  contracted per instruction, 2x PE throughput): the per-core x tile
  [128, nt, c] is the *stationary* operand (graph-major, j on partitions) and
  transposed support tiles [128, 2, 512] stream as the *moving* operand,
  producing diffused mats feature-major [c, n] in PSUM. Chain mats (x1, x3)
  are additionally PE-transposed back to graph-major fp8 for the next
  diffusion step. Terminal/all mats evacuate bf16 to a K-packed DRAM stash
  [264, n] per batch (m1..4 hx rows 0:256, m1..4 input rows 256:264), so the
  output projection is 3 accumulating matmuls (K-tiles 128/128/74) per
  (batch, n-chunk); the m0 term reads host-provided feature-major x0 (gconv1)
  or the r*hx stash (gconv2) directly. Column order puts the 512 hx feature
  columns first and the 16 input columns last: gconv2 skips input-feature
  diffusion entirely (inputs don't change between gconvs) and reuses gconv1's
  input stash rows. Chebyshev combine (x2 = 2 s x1 - x0) is folded into the
  projection weights host-side.
"""
import sys

import ml_dtypes
import numpy as np

sys.path.insert(0, "/opt/trn_rl_repo")

from concourse import bacc, mybir, tile  # noqa: E402
from concourse.bass_utils import run_bass_kernel_spmd  # noqa: E402

B = 64
N = 4096
U = 64
IN_DIM = 2
F = U + IN_DIM          # 66
NCORES = 8
BL = B // NCORES        # 8
FBH = U * BL            # 512 hx columns (c = b*64 + f)
FBI = IN_DIM * BL       # 16 input columns (c = 512 + b*2 + fi)
FB = FBH + FBI          # 528
NT = N // 128           # 32 j-tiles
JP = NT // 2            # 16 j-tile pairs (DoubleRow)
NCH = 8                 # n-chunks of 512
CHW = N // NCH          # 512
CTS_G1 = [(0, 128), (128, 128), (256, 128), (384, 128), (512, 16)]
CTS_G2 = [(0, 128), (128, 128), (256, 128), (384, 128)]
KR = 264                # stash rows per batch: 4*64 hx + 4*2 input

F32 = mybir.dt.float32
BF16 = mybir.dt.bfloat16
FP8 = mybir.dt.float8e4
NP_FP8 = ml_dtypes.float8_e4m3
DR = mybir.MatmulPerfMode.DoubleRow
SIGMOID = mybir.ActivationFunctionType.Sigmoid
TANH = mybir.ActivationFunctionType.Tanh

_BUILD_CACHE = {}
_NAME_N = [0]


def _nm(base):
    _NAME_N[0] += 1
    return f"{base}_{_NAME_N[0]}"


def _emit_spmm(nc, pools, sT_d, sup, src, cts, m_idx, stash_l, xg_dst, ident,
               post_nchunk=None):
    """One diffusion step: feature-major psum tiles of (supports[sup] @ src).

    src: graph-major fp8 SBUF tile [128, NT, fb] (stationary operand, j-pairs
    contracted via DoubleRow). Evacuates bf16 to the K-packed stash rows for
    mat m_idx; if xg_dst is given (chain mat), also PE-transposes back to
    graph-major fp8 for the next diffusion step.
    """
    pps, pst, pev = pools["pps"], pools["pst"], pools["pev"]
    for nch in range(NCH):
        n0 = nch * CHW
        psums = [pps.tile([128, CHW], F32, tag="ps", name=_nm("sp"))
                 for _ in cts]
        for jp in range(JP):
            st = pst.tile([128, 2, CHW], FP8, tag="st", name=_nm("st"))
            nc.sync.dma_start(
                st[:],
                sT_d[sup, jp * 256:(jp + 1) * 256, n0:n0 + CHW].rearrange(
                    "(t p) n -> p t n", p=128
                ),
            )
            for ci, (c0, cw) in enumerate(cts):
                nc.tensor.matmul(
                    psums[ci][0:cw, :],
                    src[:, 2 * jp:2 * jp + 2, c0:c0 + cw],
                    st[:],
                    start=(jp == 0),
                    stop=(jp == JP - 1),
                    perf_mode=DR,
                )
        for ci, (c0, cw) in enumerate(cts):
            ev = pev.tile([128, CHW], BF16, tag="ev", name=_nm("ev"))
            if ci % 2 == 0:
                nc.vector.tensor_copy(ev[0:cw, :], psums[ci][0:cw, :])
            else:
                nc.scalar.copy(ev[0:cw, :], psums[ci][0:cw, :])
            if cw == 128:
                b0 = c0 // 64
                r0 = (m_idx - 1) * 64
                nc.sync.dma_start(stash_l[nch][b0, r0:r0 + 64, :], ev[0:64, :])
                nc.sync.dma_start(
                    stash_l[nch][b0 + 1, r0:r0 + 64, :], ev[64:128, :]
                )
            else:  # input-feature tile [16, CHW]
                r0 = 256 + (m_idx - 1) * 2
                for b in range(BL):
                    nc.sync.dma_start(
                        stash_l[nch][b, r0:r0 + 2, :], ev[2 * b:2 * b + 2, :]
                    )
            if xg_dst is not None:
                for blk in range(4):
                    tp = pps.tile([128, 128], BF16, tag="tp", bufs=1,
                                  name=_nm("tp"))
                    nc.tensor.transpose(
                        tp[0:128, 0:cw],
                        ev[0:cw, blk * 128:(blk + 1) * 128],
                        ident[0:cw, 0:cw],
                    )
                    nc.scalar.copy(
                        xg_dst[:, nch * 4 + blk, c0:c0 + cw], tp[0:128, 0:cw]
                    )
        if post_nchunk is not None:
            post_nchunk(nch)


def _emit_proj_nchunk(nc, pools, g, nch, w, bias, x0f_d, stash_l, stash1_l,
                      xt02_l, u_l, out_d, xg2, ident):
    """Projection + gating for gconv g at one n-chunk (all 8 local batches)."""
    pps, pkt, psig, pgate = (
        pools["pps"], pools["pkt"], pools["psig"], pools["pgate"],
    )
    O = 128 if g == 0 else 64
    n0 = nch * CHW
    for b in range(BL):
        kt0 = pkt.tile([128, CHW], BF16, tag="kt", name=_nm("kt0"))
        nc.sync.dma_start(kt0[:], stash_l[nch][b, 0:128, :])
        kt1 = pkt.tile([128, CHW], BF16, tag="kt", name=_nm("kt1"))
        nc.sync.dma_start(kt1[:], stash_l[nch][b, 128:256, :])
        kt2 = pkt.tile([74, CHW], BF16, tag="kt2", name=_nm("kt2"))
        if g == 0:
            nc.sync.dma_start(kt2[0:64, :], x0f_d[b, 0:64, n0:n0 + CHW])
        else:
            nc.sync.dma_start(kt2[0:64, :], xt02_l[nch][b, :, :])
        nc.sync.dma_start(kt2[64:72, :], stash1_l[nch][b, 256:264, :])
        nc.sync.dma_start(kt2[72:74, :], x0f_d[b, 64:66, n0:n0 + CHW])
        pp = pps.tile([128, CHW], F32, tag="ps", name=_nm("pp"))
        nc.tensor.matmul(pp[0:O, :], w[:, 0, :], kt0[:], start=True, stop=False)
        nc.tensor.matmul(pp[0:O, :], w[:, 1, :], kt1[:], start=False, stop=False)
        nc.tensor.matmul(pp[0:O, :], w[0:74, 2, :], kt2[:], start=False,
                         stop=True)
        if g == 0:
            sig = psig.tile([128, CHW], F32, tag="sig", name=_nm("sig"))
            nc.scalar.activation(sig[:], pp[:], SIGMOID, bias=bias[:])
            s2 = psig.tile([64, CHW], BF16, tag="s2", name=_nm("s2"))
            nc.vector.tensor_mul(s2[:], sig[0:64, :], kt2[0:64, :])
            nc.sync.dma_start(xt02_l[nch][b, :, :], s2[:])
            ub = psig.tile([64, CHW], BF16, tag="ub", name=_nm("ub"))
            nc.scalar.copy(ub[:], sig[64:128, :])
            nc.sync.dma_start(u_l[nch][b, :, :], ub[:])
            for blk in range(4):
                tp = pps.tile([128, 128], BF16, tag="tp", bufs=1, name=_nm("tp"))
                nc.tensor.transpose(
                    tp[0:128, 0:64], s2[:, blk * 128:(blk + 1) * 128],
                    ident[0:64, 0:64],
                )
                nc.scalar.copy(
                    xg2[:, nch * 4 + blk, b * 64:(b + 1) * 64], tp[0:128, 0:64]
                )
        else:
            ct = pgate.tile([64, CHW], F32, tag="ct", name=_nm("ct"))
            nc.scalar.activation(ct[:], pp[0:64, :], TANH, bias=bias[:])
            ut = pgate.tile([64, CHW], BF16, tag="ut", name=_nm("ut"))
            nc.sync.dma_start(ut[:], u_l[nch][b, :, :])
            hxt = pgate.tile([64, CHW], BF16, tag="hxt", name=_nm("hxt"))
            nc.sync.dma_start(hxt[:], x0f_d[b, 0:64, n0:n0 + CHW])
            t1 = pgate.tile([64, CHW], F32, tag="t1", name=_nm("t1"))
            nc.vector.tensor_sub(t1[:], hxt[:], ct[:])
            nc.vector.tensor_mul(t1[:], ut[:], t1[:])
            nc.vector.tensor_add(t1[:], t1[:], ct[:])
            nc.sync.dma_start(out_d[b, :, n0:n0 + CHW], t1[:])


def _build(reps=1):
    if reps in _BUILD_CACHE:
        return _BUILD_CACHE[reps]
    nc = bacc.Bacc("TRN2", target_bir_lowering=False, debug=False)

    sT_d = nc.dram_tensor("sT", [2, N, N], FP8, kind="ExternalInput").ap()
    xg0_d = nc.dram_tensor("xg0", [N, FB], FP8, kind="ExternalInput").ap()
    x0f_d = nc.dram_tensor("x0f", [BL, F, N], BF16, kind="ExternalInput").ap()
    w1_d = nc.dram_tensor("w1", [128, 3, 2 * U], BF16, kind="ExternalInput").ap()
    b1_d = nc.dram_tensor("b1", [2 * U, 1], F32, kind="ExternalInput").ap()
    w2_d = nc.dram_tensor("w2", [128, 3, U], BF16, kind="ExternalInput").ap()
    b2_d = nc.dram_tensor("b2", [U, 1], F32, kind="ExternalInput").ap()
    id_d = nc.dram_tensor("ident", [128, 128], BF16, kind="ExternalInput").ap()
    out_d = nc.dram_tensor("outT", [BL, U, N], F32, kind="ExternalOutput").ap()

    with tile.TileContext(nc) as tc:
        with (
            tc.tile_pool(name="dram", bufs=1, space="DRAM") as dram,
            tc.tile_pool(name="pw", bufs=1) as pw,
            tc.tile_pool(name="pxg", bufs=1) as pxg,
            tc.tile_pool(name="pst", bufs=6) as pst,
            tc.tile_pool(name="pev", bufs=6) as pev,
            tc.tile_pool(name="pkt", bufs=6) as pkt,
            tc.tile_pool(name="psig", bufs=3) as psig,
            tc.tile_pool(name="pgate", bufs=3) as pgate,
            tc.tile_pool(name="pps", bufs=7, space="PSUM") as pps,
        ):
            pools = dict(pps=pps, pst=pst, pev=pev, pkt=pkt, psig=psig,
                         pgate=pgate)
            stash1 = [dram.tile([BL, KR, CHW], BF16, name=_nm("stA"))
                      for _ in range(NCH)]
            stash2 = [dram.tile([BL, 256, CHW], BF16, name=_nm("stB"))
                      for _ in range(NCH)]
            xt02_l = [dram.tile([BL, 64, CHW], BF16, name=_nm("xt02"))
                      for _ in range(NCH)]
            u_l = [dram.tile([BL, 64, CHW], BF16, name=_nm("ud"))
                   for _ in range(NCH)]

            w1 = pw.tile([128, 3, 2 * U], BF16, tag="w1")
            nc.sync.dma_start(w1[:], w1_d)
            w2 = pw.tile([128, 3, U], BF16, tag="w2")
            nc.sync.dma_start(w2[:], w2_d)
            b1 = pw.tile([2 * U, 1], F32, tag="b1")
            nc.sync.dma_start(b1[:], b1_d)
            b2 = pw.tile([U, 1], F32, tag="b2")
            nc.sync.dma_start(b2[:], b2_d)
            ident = pw.tile([128, 128], BF16, tag="ident")
            nc.sync.dma_start(ident[:], id_d)

            for _rep in range(reps):
                xg0 = pxg.tile([128, NT, FB], FP8, tag="xg", bufs=3,
                               name=_nm("xg0"))
                nc.sync.dma_start(
                    xg0[:], xg0_d.rearrange("(t p) c -> p t c", p=128)
                )

                # ---- gconv 1 diffusion (528 cols: hx + inputs) ----
                xg1 = pxg.tile([128, NT, FB], FP8, tag="xg", bufs=3,
                               name=_nm("xg1"))
                _emit_spmm(nc, pools, sT_d, 0, xg0, CTS_G1, 1, stash1, xg1,
                           ident)
                _emit_spmm(nc, pools, sT_d, 0, xg1, CTS_G1, 2, stash1, None,
                           ident)
                xg3 = pxg.tile([128, NT, FB], FP8, tag="xg", bufs=3,
                               name=_nm("xg3"))
                _emit_spmm(nc, pools, sT_d, 1, xg0, CTS_G1, 3, stash1, xg3,
                           ident)

                # ---- x4 + gconv1 projection interleaved per n-chunk ----
                xg2 = pxg.tile([128, NT, FBH], FP8, tag="xg2", bufs=2,
                               name=_nm("xg2"))

                def proj1(nch, _xg2=xg2):
                    _emit_proj_nchunk(
                        nc, pools, 0, nch, w1, b1, x0f_d, stash1, stash1,
                        xt02_l, u_l, out_d, _xg2, ident,
                    )

                _emit_spmm(nc, pools, sT_d, 1, xg3, CTS_G1, 4, stash1, None,
                           ident, post_nchunk=proj1)

                # ---- gconv 2 diffusion (512 hx cols only) ----
                xg1b = pxg.tile([128, NT, FBH], FP8, tag="xg2", bufs=2,
                                name=_nm("xg1b"))
                _emit_spmm(nc, pools, sT_d, 0, xg2, CTS_G2, 1, stash2, xg1b,
                           ident)
                _emit_spmm(nc, pools, sT_d, 0, xg1b, CTS_G2, 2, stash2, None,
                           ident)
                xg3b = pxg.tile([128, NT, FBH], FP8, tag="xg2", bufs=2,
                                name=_nm("xg3b"))
                _emit_spmm(nc, pools, sT_d, 1, xg2, CTS_G2, 3, stash2, xg3b,
                           ident)

                def proj2(nch):
                    _emit_proj_nchunk(
                        nc, pools, 1, nch, w2, b2, x0f_d, stash2, stash1,
                        xt02_l, u_l, out_d, None, ident,
                    )

                _emit_spmm(nc, pools, sT_d, 1, xg3b, CTS_G2, 4, stash2, None,
                           ident, post_nchunk=proj2)

    nc.compile()
    _BUILD_CACHE[reps] = nc
    return nc


def _prep_w(w, o):
    """Fold Chebyshev combine and pack weight rows into the stash K-order.

    Reference rows are (f, m) with f order [inputs(2), hx(64)]. Stash K-order:
    [m1..4 hx (256) | m0 hx (64) | m1..4 in (8) | m0 in (2)].
    """
    wr = w.reshape(F, 5, o).astype(np.float32).copy()
    wr[:, 0] = wr[:, 0] - wr[:, 2] - wr[:, 4]
    wr[:, 2] = 2.0 * wr[:, 2]
    wr[:, 4] = 2.0 * wr[:, 4]
    packed = np.zeros((330, o), np.float32)
    for m in range(1, 5):
        packed[(m - 1) * 64:m * 64] = wr[IN_DIM:, m]
        packed[320 + (m - 1) * 2:320 + m * 2] = wr[:IN_DIM, m]
    packed[256:320] = wr[IN_DIM:, 0]
    packed[328:330] = wr[:IN_DIM, 0]
    wsb = np.zeros((128, 3, o), np.float32)
    wsb[:, 0] = packed[0:128]
    wsb[:, 1] = packed[128:256]
    wsb[0:74, 2] = packed[256:330]
    return np.ascontiguousarray(wsb).astype(ml_dtypes.bfloat16)


def _host_prep(inputs, hx, supports, ru_weights, ru_biases, gconv_weights,
               gconv_biases):
    """Build per-core input maps."""
    sT = np.ascontiguousarray(
        np.asarray(supports).transpose(0, 2, 1)
    ).astype(NP_FP8)

    w1 = _prep_w(np.asarray(ru_weights), 2 * U)
    w2 = _prep_w(np.asarray(gconv_weights), U)
    b1 = np.ascontiguousarray(
        np.asarray(ru_biases).reshape(2 * U, 1)
    ).astype(np.float32)
    b2 = np.ascontiguousarray(
        np.asarray(gconv_biases).reshape(U, 1)
    ).astype(np.float32)
    ident = np.eye(128, dtype=ml_dtypes.bfloat16)

    hx_r = np.asarray(hx, np.float32).reshape(B, N, U)
    in_r = np.asarray(inputs, np.float32).reshape(B, N, IN_DIM)

    in_maps = []
    for c in range(NCORES):
        hxc = hx_r[c * BL:(c + 1) * BL]   # [BL, N, 64]
        inc = in_r[c * BL:(c + 1) * BL]   # [BL, N, 2]
        xg0 = np.concatenate(
            [
                hxc.transpose(1, 0, 2).reshape(N, FBH),
                inc.transpose(1, 0, 2).reshape(N, FBI),
            ],
            axis=1,
        ).astype(NP_FP8)
        x0f = np.concatenate(
            [hxc.transpose(0, 2, 1), inc.transpose(0, 2, 1)], axis=1
        ).astype(ml_dtypes.bfloat16)  # [BL, 66, N]
        in_maps.append({
            "sT": sT, "xg0": np.ascontiguousarray(xg0),
            "x0f": np.ascontiguousarray(x0f),
            "w1": w1, "b1": b1, "w2": w2, "b2": b2, "ident": ident,
        })
    return in_maps


def kernel(inputs, hx, supports, ru_weights, ru_biases, gconv_weights,
           gconv_biases):
    nc = _build()
    in_maps = _host_prep(
        inputs, hx, supports, ru_weights, ru_biases, gconv_weights,
        gconv_biases
    )
    res = run_bass_kernel_spmd(nc, in_maps, list(range(NCORES))).results
    outs = []
    for c in range(NCORES):
        outT = res[c]["outT"]  # [BL, U, N]
        outs.append(outT.transpose(0, 2, 1).reshape(BL, N * U))
    return np.concatenate(outs, axis=0).astype(np.float32)


# revision 8
# speedup vs baseline: 71.2852x; 59.9035x over previous
"""DCGRU cell on 8 Trainium2 NeuronCores — fp8 DoubleRow spmm version.

Sharding: data-parallel over batch B=64 -> 8 batches per core. Supports and
weights replicated; everything per-core is local (no collectives).

Per-core computation (Bl=8 local batches, N=4096 nodes, F=66 feats):
  The 8 graph-diffusion spmms run as fp8-e4m3 DoubleRow matmuls (two j-tiles
  contracted per instruction, 2x PE throughput): the per-core x tile
  [128, nt, c] is the *stationary* operand (graph-major, j on partitions) and
  transposed support tiles [128, 4, 512] stream as the *moving* operand,
  producing diffused mats feature-major [c, n] in PSUM. Chain mats (x1, x3)
  are additionally PE-transposed back to graph-major fp8 for the next
  diffusion step. All mats evacuate bf16 to a K-packed DRAM stash
  [256, n] per batch (m1..4 hx rows) + a tiny input-feature stash, so the
  output projection is 3 accumulating matmuls (K-tiles 128/128/74) per
  (batch, n-chunk); the m0 term reads host-provided feature-major x0 (gconv1)
  or the r*hx stash (gconv2) directly. Column order puts the 512 hx feature
  columns first and the 16 input columns last: gconv2 skips input-feature
  diffusion entirely (inputs don't change between gconvs) and reuses gconv1's
  input stash. Chebyshev combine (x2 = 2 s x1 - x0) is folded into the
  projection weights host-side.

  Projection kt loads are issued one n-chunk ahead of their compute so the
  DMA transfers hide under the next spmm sweep. DMA issue work is split
  across the sync (supports + DRAM writes) and scalar (proj/gate loads) HW
  DGE queues; SBUF-only elementwise gate math runs on the idle GpSimd.
"""
import sys

import ml_dtypes
import numpy as np

sys.path.insert(0, "/opt/trn_rl_repo")

from concourse import bacc, mybir, tile  # noqa: E402
from concourse.bass_utils import run_bass_kernel_spmd  # noqa: E402

B = 64
N = 4096
U = 64
IN_DIM = 2
F = U + IN_DIM          # 66
NCORES = 8
BL = B // NCORES        # 8
FBH = U * BL            # 512 hx columns (c = b*64 + f)
FBI = IN_DIM * BL       # 16 input columns (c = 512 + b*2 + fi)
FB = FBH + FBI          # 528
NT = N // 128           # 32 j-tiles
JQ = NT // 4            # 8 j-quads (two DoubleRow pairs per support load)
NCH = 8                 # n-chunks of 512
CHW = N // NCH          # 512
CTS_G1 = [(0, 128), (128, 128), (256, 128), (384, 128), (512, 16)]
CTS_G2 = [(0, 128), (128, 128), (256, 128), (384, 128)]

F32 = mybir.dt.float32
BF16 = mybir.dt.bfloat16
FP8 = mybir.dt.float8e4
NP_FP8 = ml_dtypes.float8_e4m3
DR = mybir.MatmulPerfMode.DoubleRow
SIGMOID = mybir.ActivationFunctionType.Sigmoid
TANH = mybir.ActivationFunctionType.Tanh

_BUILD_CACHE = {}
_NAME_N = [0]


def _nm(base):
    _NAME_N[0] += 1
    return f"{base}_{_NAME_N[0]}"


def _emit_spmm(nc, pools, sT_d, sup, src, cts, m_idx, stash_l, instash_l,
               xg_dst, ident, post_evac=None):
    """One diffusion step: feature-major psum tiles of (supports[sup] @ src).

    src: graph-major fp8 SBUF tile [128, NT, fb] (stationary operand, j-pairs
    contracted via DoubleRow). Evacuates bf16 to the K-packed stash for mat
    m_idx; if xg_dst is given (chain mat), also PE-transposes back to
    graph-major fp8. post_evac(nch) is called right after nch's evacuations.
    """
    pps, pst, pev = pools["pps"], pools["pst"], pools["pev"]
    for nch in range(NCH):
        n0 = nch * CHW
        psums = []
        for ci, (c0, cw) in enumerate(cts):
            tag = "pp" if cw == 16 else "ps"
            psums.append(pps.tile([128, CHW], F32, tag=tag,
                                  bufs=(2 if tag == "pp" else 4),
                                  name=_nm("sp")))
        for jq in range(JQ):
            st = pst.tile([128, 4, CHW], FP8, tag="st", bufs=4, name=_nm("st"))
            nc.sync.dma_start(
                st[:],
                sT_d[sup, jq * 512:(jq + 1) * 512, n0:n0 + CHW].rearrange(
                    "(t p) n -> p t n", p=128
                ),
            )
            for k in range(2):
                jp = jq * 2 + k
                for ci, (c0, cw) in enumerate(cts):
                    nc.tensor.matmul(
                        psums[ci][0:cw, :],
                        src[:, 2 * jp:2 * jp + 2, c0:c0 + cw],
                        st[:, 2 * k:2 * k + 2, :],
                        start=(jp == 0),
                        stop=(jp == 2 * JQ - 1),
                        perf_mode=DR,
                    )
        for ci, (c0, cw) in enumerate(cts):
            ev = pev.tile([128, CHW], BF16, tag="ev", name=_nm("ev"))
            nc.vector.tensor_copy(ev[0:cw, :], psums[ci][0:cw, :])
            if cw == 128:
                b0 = c0 // 64
                r0 = (m_idx - 1) * 64
                nc.sync.dma_start(
                    stash_l[nch][b0:b0 + 2, r0:r0 + 64, :], ev[:, :]
                )
            else:  # input-feature tile [16, CHW] -> instash slot m_idx-1
                nc.sync.dma_start(
                    instash_l[nch][:, m_idx - 1, :], ev[0:16, :]
                )
            if xg_dst is not None:
                for blk in range(4):
                    tp = pps.tile([128, 128], BF16, tag="tp", bufs=2,
                                  name=_nm("tp"))
                    nc.tensor.transpose(
                        tp[0:128, 0:cw],
                        ev[0:cw, blk * 128:(blk + 1) * 128],
                        ident[0:cw, 0:cw],
                    )
                    nc.scalar.copy(
                        xg_dst[:, nch * 4 + blk, c0:c0 + cw],
                        tp[0:128, 0:cw],
                    )
        if post_evac is not None:
            post_evac(nch)


def _proj_loads(nc, pools, g, nch, x0f_d, stash_l, instash_l, xt02_l, u_l):
    """Issue the DMA loads for projection/gating of n-chunk nch (both gconvs:
    4 two-batch groups). Returns the tiles for _proj_compute."""
    pkt = pools["pkt"]
    n0 = nch * CHW
    groups = []
    for b0 in range(0, BL, 2):
        ktt = pkt.tile([128, 4, CHW], BF16, tag="ktt", bufs=6, name=_nm("ktt"))
        nc.scalar.dma_start(
            ktt[:],
            stash_l[nch][b0:b0 + 2, 0:256, :].rearrange(
                "b (t p) n -> p (b t) n", p=128
            ),
        )
        kt2 = pkt.tile([74, 2, CHW], BF16, tag="kt2", bufs=6, name=_nm("kt2"))
        if g == 0:
            nc.scalar.dma_start(
                kt2[0:64, :, :],
                x0f_d[b0:b0 + 2, 0:64, n0:n0 + CHW].rearrange(
                    "b f n -> f b n"
                ),
            )
        else:
            nc.scalar.dma_start(
                kt2[0:64, :, :],
                xt02_l[nch][b0:b0 + 2, :, :].rearrange("b f n -> f b n"),
            )
        nc.scalar.dma_start(
            kt2[64:74, :, :],
            instash_l[nch][2 * b0:2 * b0 + 4, :, :].rearrange(
                "(db fi) s n -> (fi s) db n", db=2
            ),
        )
        grp = {"ktt": ktt, "kt2": kt2}
        if g == 1:
            ut = pkt.tile([64, 2, CHW], BF16, tag="ut", bufs=4, name=_nm("ut"))
            nc.scalar.dma_start(
                ut[:], u_l[nch][b0:b0 + 2, :, :].rearrange("b f n -> f b n")
            )
            hxt = pkt.tile([64, 2, CHW], BF16, tag="hxt", bufs=4,
                           name=_nm("hxt"))
            nc.scalar.dma_start(
                hxt[:],
                x0f_d[b0:b0 + 2, 0:64, n0:n0 + CHW].rearrange("b f n -> f b n"),
            )
            grp["ut"] = ut
            grp["hxt"] = hxt
        groups.append(grp)
    return groups


def _proj_compute(nc, pools, g, nch, groups, w, bias, xt02_l, u_l, out_d,
                  xg2, ident):
    """Projection matmuls + gating for n-chunk nch (loads already issued)."""
    pps, psig, pgate = pools["pps"], pools["psig"], pools["pgate"]
    O = 128 if g == 0 else 64
    n0 = nch * CHW
    for gi, grp in enumerate(groups):
        b0 = gi * 2
        ktt, kt2 = grp["ktt"], grp["kt2"]
        if g == 0:
            s2v = psig.tile([64, 2, CHW], BF16, tag="s2", name=_nm("s2"))
            ubv = psig.tile([64, 2, CHW], BF16, tag="ub", name=_nm("ub"))
        else:
            ctv = pgate.tile([64, 2, CHW], F32, tag="ct", name=_nm("ct"))
            t1v = pgate.tile([64, 2, CHW], F32, tag="t1", name=_nm("t1"))
        for db in range(2):
            pp = pps.tile([128, CHW], F32, tag="pp", bufs=2, name=_nm("pp"))
            nc.tensor.matmul(pp[0:O, :], w[:, 0, :], ktt[:, 2 * db, :],
                             start=True, stop=False)
            nc.tensor.matmul(pp[0:O, :], w[:, 1, :], ktt[:, 2 * db + 1, :],
                             start=False, stop=False)
            nc.tensor.matmul(pp[0:O, :], w[0:74, 2, :], kt2[:, db, :],
                             start=False, stop=True)
            if g == 0:
                sig = psig.tile([128, CHW], F32, tag="sig", name=_nm("sig"))
                nc.scalar.activation(sig[:], pp[:], SIGMOID, bias=bias[:])
                nc.gpsimd.tensor_mul(
                    s2v[:, db, :], sig[0:64, :], kt2[0:64, db, :]
                )
                nc.gpsimd.tensor_copy(ubv[:, db, :], sig[64:128, :])
                for blk in range(4):
                    tp = pps.tile([128, 128], BF16, tag="tp", bufs=2,
                                  name=_nm("tp"))
                    nc.tensor.transpose(
                        tp[0:128, 0:64],
                        s2v[:, db, blk * 128:(blk + 1) * 128],
                        ident[0:64, 0:64],
                    )
                    nc.scalar.copy(
                        xg2[:, nch * 4 + blk, (b0 + db) * 64:(b0 + db + 1) * 64],
                        tp[0:128, 0:64],
                    )
            else:
                nc.scalar.activation(ctv[:, db, :], pp[0:64, :], TANH,
                                     bias=bias[:])
                ut, hxt = grp["ut"], grp["hxt"]
                nc.gpsimd.tensor_sub(t1v[:, db, :], hxt[:, db, :],
                                     ctv[:, db, :])
                nc.gpsimd.tensor_mul(t1v[:, db, :], ut[:, db, :],
                                     t1v[:, db, :])
                nc.gpsimd.tensor_add(t1v[:, db, :], t1v[:, db, :],
                                     ctv[:, db, :])
        if g == 0:
            nc.sync.dma_start(
                xt02_l[nch][b0:b0 + 2, :, :].rearrange("b f n -> f b n"),
                s2v[:, :, :],
            )
            nc.sync.dma_start(
                u_l[nch][b0:b0 + 2, :, :].rearrange("b f n -> f b n"),
                ubv[:, :, :],
            )
        else:
            nc.sync.dma_start(
                out_d[b0:b0 + 2, :, n0:n0 + CHW].rearrange("b u n -> u b n"),
                t1v[:, :, :],
            )


def _build(reps=1):
    if reps in _BUILD_CACHE:
        return _BUILD_CACHE[reps]
    nc = bacc.Bacc("TRN2", target_bir_lowering=False, debug=False)

    sT_d = nc.dram_tensor("sT", [2, N, N], FP8, kind="ExternalInput").ap()
    xg0_d = nc.dram_tensor("xg0", [N, FB], FP8, kind="ExternalInput").ap()
    x0f_d = nc.dram_tensor("x0f", [BL, F, N], BF16, kind="ExternalInput").ap()
    w1_d = nc.dram_tensor("w1", [128, 3, 2 * U], BF16, kind="ExternalInput").ap()
    b1_d = nc.dram_tensor("b1", [2 * U, 1], F32, kind="ExternalInput").ap()
    w2_d = nc.dram_tensor("w2", [128, 3, U], BF16, kind="ExternalInput").ap()
    b2_d = nc.dram_tensor("b2", [U, 1], F32, kind="ExternalInput").ap()
    id_d = nc.dram_tensor("ident", [128, 128], BF16, kind="ExternalInput").ap()
    out_d = nc.dram_tensor("outT", [BL, U, N], F32, kind="ExternalOutput").ap()

    with tile.TileContext(nc) as tc:
        with (
            tc.tile_pool(name="dram", bufs=1, space="DRAM") as dram,
            tc.tile_pool(name="pw", bufs=1) as pw,
            tc.tile_pool(name="pxg", bufs=1) as pxg,
            tc.tile_pool(name="pst", bufs=4) as pst,
            tc.tile_pool(name="pev", bufs=6) as pev,
            tc.tile_pool(name="pkt", bufs=6) as pkt,
            tc.tile_pool(name="psig", bufs=3) as psig,
            tc.tile_pool(name="pgate", bufs=3) as pgate,
            tc.tile_pool(name="pps", bufs=4, space="PSUM") as pps,
        ):
            pools = dict(pps=pps, pst=pst, pev=pev, pkt=pkt, psig=psig,
                         pgate=pgate)
            stash1 = [dram.tile([BL, 256, CHW], BF16, name=_nm("stA"))
                      for _ in range(NCH)]
            stash2 = [dram.tile([BL, 256, CHW], BF16, name=_nm("stB"))
                      for _ in range(NCH)]
            # input-feature stash: [16=(b,fi), slot, CHW]; slots 0..3 = m1..4
            # (written by gconv1 diffusion), slot 4 = m0 (filled once from x0f)
            instash = [dram.tile([16, 5, CHW], BF16, name=_nm("stI"))
                       for _ in range(NCH)]
            xt02_l = [dram.tile([BL, 64, CHW], BF16, name=_nm("xt02"))
                      for _ in range(NCH)]
            u_l = [dram.tile([BL, 64, CHW], BF16, name=_nm("ud"))
                   for _ in range(NCH)]

            w1 = pw.tile([128, 3, 2 * U], BF16, tag="w1")
            nc.sync.dma_start(w1[:], w1_d)
            w2 = pw.tile([128, 3, U], BF16, tag="w2")
            nc.sync.dma_start(w2[:], w2_d)
            b1 = pw.tile([2 * U, 1], F32, tag="b1")
            nc.sync.dma_start(b1[:], b1_d)
            b2 = pw.tile([U, 1], F32, tag="b2")
            nc.sync.dma_start(b2[:], b2_d)
            ident = pw.tile([128, 128], BF16, tag="ident")
            nc.sync.dma_start(ident[:], id_d)
            for nch in range(NCH):
                nc.sync.dma_start(
                    instash[nch][:, 4, :].rearrange("(b f) n -> b f n", b=8),
                    x0f_d[:, 64:66, nch * CHW:(nch + 1) * CHW],
                )

            for _rep in range(reps):
                xg0 = pxg.tile([128, NT, FB], FP8, tag="xg", bufs=3,
                               name=_nm("xg0"))
                nc.sync.dma_start(
                    xg0[:], xg0_d.rearrange("(t p) c -> p t c", p=128)
                )

                def make_post(g, stash_l, w, bias, xg2):
                    state = {"groups": None}

                    def post_evac(nch, _g=g, _st=stash_l, _w=w, _b=bias,
                                  _xg2=xg2, _state=state):
                        if nch > 0:
                            _proj_compute(nc, pools, _g, nch - 1,
                                          _state["groups"], _w, _b, xt02_l,
                                          u_l, out_d, _xg2, ident)
                        _state["groups"] = _proj_loads(
                            nc, pools, _g, nch, x0f_d, _st, instash, xt02_l,
                            u_l)
                        if nch == NCH - 1:
                            _proj_compute(nc, pools, _g, nch,
                                          _state["groups"], _w, _b, xt02_l,
                                          u_l, out_d, _xg2, ident)

                    return post_evac

                # ---- gconv 1 diffusion (528 cols: hx + inputs) ----
                xg1 = pxg.tile([128, NT, FB], FP8, tag="xg", bufs=3,
                               name=_nm("xg1"))
                _emit_spmm(nc, pools, sT_d, 0, xg0, CTS_G1, 1, stash1,
                           instash, xg1, ident)
                _emit_spmm(nc, pools, sT_d, 0, xg1, CTS_G1, 2, stash1,
                           instash, None, ident)
                xg3 = pxg.tile([128, NT, FB], FP8, tag="xg", bufs=3,
                               name=_nm("xg3"))
                _emit_spmm(nc, pools, sT_d, 1, xg0, CTS_G1, 3, stash1,
                           instash, xg3, ident)
                xg2 = pxg.tile([128, NT, FBH], FP8, tag="xg2", bufs=2,
                               name=_nm("xg2"))
                _emit_spmm(nc, pools, sT_d, 1, xg3, CTS_G1, 4, stash1,
                           instash, None, ident,
                           post_evac=make_post(0, stash1, w1, b1, xg2))

                # ---- gconv 2 diffusion (512 hx cols only) ----
                xg1b = pxg.tile([128, NT, FBH], FP8, tag="xg2", bufs=2,
                                name=_nm("xg1b"))
                _emit_spmm(nc, pools, sT_d, 0, xg2, CTS_G2, 1, stash2,
                           instash, xg1b, ident)
                _emit_spmm(nc, pools, sT_d, 0, xg1b, CTS_G2, 2, stash2,
                           instash, None, ident)
                xg3b = pxg.tile([128, NT, FBH], FP8, tag="xg2", bufs=2,
                                name=_nm("xg3b"))
                _emit_spmm(nc, pools, sT_d, 1, xg2, CTS_G2, 3, stash2,
                           instash, xg3b, ident)
                _emit_spmm(nc, pools, sT_d, 1, xg3b, CTS_G2, 4, stash2,
                           instash, None, ident,
                           post_evac=make_post(1, stash2, w2, b2, None))

    nc.compile()
    _BUILD_CACHE[reps] = nc
    return nc


def _prep_w(w, o):
    """Fold Chebyshev combine and pack weight rows into the stash K-order.

    Reference rows are (f, m) with f order [inputs(2), hx(64)]. Stash K-order:
    [m1..4 hx (256) | m0 hx (64) | (fi, slot) with slots m1..4,m0 (10)].
    """
    wr = w.reshape(F, 5, o).astype(np.float32).copy()
    wr[:, 0] = wr[:, 0] - wr[:, 2] - wr[:, 4]
    wr[:, 2] = 2.0 * wr[:, 2]
    wr[:, 4] = 2.0 * wr[:, 4]
    packed = np.zeros((330, o), np.float32)
    for m in range(1, 5):
        packed[(m - 1) * 64:m * 64] = wr[IN_DIM:, m]
    packed[256:320] = wr[IN_DIM:, 0]
    for fi in range(IN_DIM):
        for si, m in enumerate([1, 2, 3, 4, 0]):
            packed[320 + fi * 5 + si] = wr[fi, m]
    wsb = np.zeros((128, 3, o), np.float32)
    wsb[:, 0] = packed[0:128]
    wsb[:, 1] = packed[128:256]
    wsb[0:74, 2] = packed[256:330]
    return np.ascontiguousarray(wsb).astype(ml_dtypes.bfloat16)


def _host_prep(inputs, hx, supports, ru_weights, ru_biases, gconv_weights,
               gconv_biases):
    """Build per-core input maps."""
    sT = np.ascontiguousarray(
        np.asarray(supports).transpose(0, 2, 1)
    ).astype(NP_FP8)

    w1 = _prep_w(np.asarray(ru_weights), 2 * U)
    w2 = _prep_w(np.asarray(gconv_weights), U)
    b1 = np.ascontiguousarray(
        np.asarray(ru_biases).reshape(2 * U, 1)
    ).astype(np.float32)
    b2 = np.ascontiguousarray(
        np.asarray(gconv_biases).reshape(U, 1)
    ).astype(np.float32)
    ident = np.eye(128, dtype=ml_dtypes.bfloat16)

    hx_r = np.asarray(hx, np.float32).reshape(B, N, U)
    in_r = np.asarray(inputs, np.float32).reshape(B, N, IN_DIM)

    in_maps = []
    for c in range(NCORES):
        hxc = hx_r[c * BL:(c + 1) * BL]   # [BL, N, 64]
        inc = in_r[c * BL:(c + 1) * BL]   # [BL, N, 2]
        xg0 = np.concatenate(
            [
                hxc.transpose(1, 0, 2).reshape(N, FBH),
                inc.transpose(1, 0, 2).reshape(N, FBI),
            ],
            axis=1,
        ).astype(NP_FP8)
        x0f = np.concatenate(
            [hxc.transpose(0, 2, 1), inc.transpose(0, 2, 1)], axis=1
        ).astype(ml_dtypes.bfloat16)  # [BL, 66, N]
        in_maps.append({
            "sT": sT, "xg0": np.ascontiguousarray(xg0),
            "x0f": np.ascontiguousarray(x0f),
            "w1": w1, "b1": b1, "w2": w2, "b2": b2, "ident": ident,
        })
    return in_maps


def kernel(inputs, hx, supports, ru_weights, ru_biases, gconv_weights,
           gconv_biases):
    nc = _build()
    in_maps = _host_prep(
        inputs, hx, supports, ru_weights, ru_biases, gconv_weights,
        gconv_biases
    )
    res = run_bass_kernel_spmd(nc, in_maps, list(range(NCORES))).results
    outs = []
    for c in range(NCORES):
        outT = res[c]["outT"]  # [BL, U, N]
        outs.append(outT.transpose(0, 2, 1).reshape(BL, N * U))
    return np.concatenate(outs, axis=0).astype(np.float32)
